# revision 1
# baseline (speedup 1.0000x reference)
import numpy as np
import jax
import jax.numpy as jnp
from functools import partial

# nn_AutoIntLayer: B=16384, F=40, E=64, H=2, 2 layers shared weights.
# Pure data parallel across 8 NeuronCores: shard batch dim of x,
# replicate the tiny [64,64] projections + LN params.

LAYER_NUM = 2
HEAD_NUM = 2
LN_EPS = 1e-3
N_CORES = 8


def _split_heads(t, H):
    B, F, E = t.shape
    return t.reshape(B, F, H, E // H).transpose(0, 2, 1, 3)


def _merge_heads(t):
    B, H, F, Dh = t.shape
    return t.transpose(0, 2, 1, 3).reshape(B, F, H * Dh)


def _layer_stack(x, Wq, bq, Wk, bk, Wv, bv, Wr, br, gamma, beta):
    H = HEAD_NUM
    out = x
    for _ in range(LAYER_NUM):
        q = jax.nn.relu(out @ Wq + bq)
        k = jax.nn.relu(out @ Wk + bk)
        v = jax.nn.relu(out @ Wv + bv)
        res = jax.nn.relu(out @ Wr + br)
        qh = _split_heads(q, H)
        kh = _split_heads(k, H)
        vh = _split_heads(v, H)
        Dh = qh.shape[-1]
        w = jnp.einsum("bhfd,bhgd->bhfg", qh, kh) / jnp.sqrt(jnp.float32(Dh))
        w = jax.nn.softmax(w, axis=-1)
        o = jnp.einsum("bhfg,bhgd->bhfd", w, vh)
        out = _merge_heads(o) + res
        out = jax.nn.relu(out)
        mu = jnp.mean(out, axis=-1, keepdims=True)
        var = jnp.mean(jnp.square(out - mu), axis=-1, keepdims=True)
        out = (out - mu) * jax.lax.rsqrt(var + LN_EPS) * gamma + beta
    return out


_pmapped = None


def _get_pmapped():
    global _pmapped
    if _pmapped is None:
        _pmapped = jax.pmap(_layer_stack, axis_name="i",
                            in_axes=(0,) + (None,) * 10)
    return _pmapped


def kernel(x, Wq, bq, Wk, bk, Wv, bv, Wr, br, gamma, beta):
    x = np.asarray(x)
    B = x.shape[0]
    assert B % N_CORES == 0
    xs = x.reshape(N_CORES, B // N_CORES, *x.shape[1:])
    f = _get_pmapped()
    out = f(xs, np.asarray(Wq), np.asarray(bq), np.asarray(Wk), np.asarray(bk),
            np.asarray(Wv), np.asarray(bv), np.asarray(Wr), np.asarray(br),
            np.asarray(gamma), np.asarray(beta))
    out = np.asarray(out).reshape(B, *x.shape[1:])
    return out.astype(np.float32)


# revision 2
# speedup vs baseline: 288.0548x; 288.0548x over previous
import numpy as np
import jax
import jax.numpy as jnp
from functools import partial

# nn_AutoIntLayer: B=16384, F=40, E=64, H=2, 2 layers shared weights.
# Pure data parallel across 8 NeuronCores: shard batch dim of x,
# replicate the tiny [64,64] projections + LN params.

LAYER_NUM = 2
HEAD_NUM = 2
LN_EPS = 1e-3
N_CORES = 8


def _split_heads(t, H):
    B, F, E = t.shape
    return t.reshape(B, F, H, E // H).transpose(0, 2, 1, 3)


def _merge_heads(t):
    B, H, F, Dh = t.shape
    return t.transpose(0, 2, 1, 3).reshape(B, F, H * Dh)


def _layer_stack(x, Wq, bq, Wk, bk, Wv, bv, Wr, br, gamma, beta):
    H = HEAD_NUM
    out = x
    for _ in range(LAYER_NUM):
        q = jax.nn.relu(out @ Wq + bq)
        k = jax.nn.relu(out @ Wk + bk)
        v = jax.nn.relu(out @ Wv + bv)
        res = jax.nn.relu(out @ Wr + br)
        qh = _split_heads(q, H)
        kh = _split_heads(k, H)
        vh = _split_heads(v, H)
        Dh = qh.shape[-1]
        w = jnp.einsum("bhfd,bhgd->bhfg", qh, kh) / jnp.sqrt(jnp.float32(Dh))
        w = jax.nn.softmax(w, axis=-1)
        o = jnp.einsum("bhfg,bhgd->bhfd", w, vh)
        out = _merge_heads(o) + res
        out = jax.nn.relu(out)
        mu = jnp.mean(out, axis=-1, keepdims=True)
        var = jnp.mean(jnp.square(out - mu), axis=-1, keepdims=True)
        out = (out - mu) * jax.lax.rsqrt(var + LN_EPS) * gamma + beta
    return out


_pmapped = None


def _get_pmapped():
    global _pmapped
    if _pmapped is None:
        _pmapped = jax.pmap(_layer_stack, axis_name="i",
                            in_axes=(0,) + (None,) * 10)
    return _pmapped


def kernel(x, Wq, bq, Wk, bk, Wv, bv, Wr, br, gamma, beta):
    x = np.asarray(x)
    B = x.shape[0]
    ws = [np.asarray(a) for a in (Wq, bq, Wk, bk, Wv, bv, Wr, br, gamma, beta)]
    try:
        n = N_CORES
        assert B % n == 0 and len(jax.devices()) >= n
        xs = x.reshape(n, B // n, *x.shape[1:])
        out = _get_pmapped()(xs, *ws)
        out = np.asarray(out).reshape(B, *x.shape[1:])
    except Exception:
        # Fallback: single-device jit on the default backend.
        out = np.asarray(jax.jit(_layer_stack)(x, *ws))
    return out.astype(np.float32)
